# revision 3
# baseline (speedup 1.0000x reference)
"""CTC best-path decoder (beam_width=1) for Trainium2, 8 NeuronCores.

Heavy part (argmax + max over the 2500-class axis of [64,256,2500] softmax
probs) runs on device, data-parallel over the batch: each core gets 8
sequences = 2048 (b,t) rows. Per 128-row tile:
  1. stream [128,2500] f32 HBM->SBUF
  2. DVE max (top-8) -> row max
  3. DVE max_index -> argmax position
Host combines, computes scores = -sum(log(maxp+eps)) and the cheap [64,256]
CTC collapse in numpy. log is monotone so argmax(p) == argmax(log(p+eps)),
and max(log(p+eps)) == log(max(p)+eps) exactly (f32 weak monotonicity).
"""

import numpy as np

import concourse.bacc as bacc
import concourse.mybir as mybir
import concourse.tile as tile
from concourse.bass_utils import run_bass_kernel_spmd

EPS = 1e-7

B, T, C = 64, 256, 2500
NCORES = 8
BLOC = B // NCORES          # sequences per core
ROWS = BLOC * T             # 2048 (b,t) rows per core
P = 128
NTILES = ROWS // P          # 16


def build():
    nc = bacc.Bacc("TRN2", target_bir_lowering=False, debug=False)

    x = nc.declare_dram_parameter("x", [ROWS, C], mybir.dt.float32, isOutput=False)
    out_val = nc.declare_dram_parameter(
        "out_val", [P, NTILES * 8], mybir.dt.float32, isOutput=True
    )
    out_ci = nc.declare_dram_parameter(
        "out_ci", [P, NTILES * 8], mybir.dt.uint32, isOutput=True
    )

    x_tiles = x[:].rearrange("(n p) c -> n p c", p=P)        # [16,128,2500]

    with tile.TileContext(nc) as tc:
        with (
            tc.tile_pool(name="xp", bufs=3) as xp,
            tc.tile_pool(name="acc", bufs=1) as accp,
        ):
            gm_all = accp.tile([P, NTILES * 8], mybir.dt.float32)
            ci_all = accp.tile([P, NTILES * 8], mybir.dt.uint32)

            for t in range(NTILES):
                xt = xp.tile([P, C], mybir.dt.float32, tag="xt")
                nc.sync.dma_start(out=xt[:], in_=x_tiles[t])

                gm = gm_all[:, t * 8 : (t + 1) * 8]
                nc.vector.max(out=gm, in_=xt[:])
                nc.vector.max_index(
                    out=ci_all[:, t * 8 : (t + 1) * 8], in_max=gm, in_values=xt[:]
                )

            nc.sync.dma_start(out=out_val[:], in_=gm_all[:])
            nc.sync.dma_start(out=out_ci[:], in_=ci_all[:])

    nc.finalize()
    return nc


def _run_device(x, trace=False):
    """x: [64,256,2500] f32 -> (best [64,256] int64, maxp [64,256] f32, results)"""
    shards = np.ascontiguousarray(x.reshape(NCORES, ROWS, C))
    nc = build()
    in_maps = [{"x": shards[i]} for i in range(NCORES)]
    res = run_bass_kernel_spmd(
        nc, in_maps, core_ids=list(range(NCORES)), trace=trace
    )
    best = np.empty((NCORES, ROWS), np.int64)
    maxp = np.empty((NCORES, ROWS), np.float32)
    for i in range(NCORES):
        gm = res.results[i]["out_val"][:, ::8]                  # [128,16]
        ci = res.results[i]["out_ci"][:, ::8].astype(np.int64)
        maxp[i] = gm.T.reshape(ROWS)
        best[i] = ci.T.reshape(ROWS)
    return best.reshape(B, T), maxp.reshape(B, T), res


def _decode(best, maxp):
    """CTC collapse + scores, numpy, matching K.ctc_decode semantics."""
    blank = C - 1
    max_lp = np.log(maxp + np.float32(EPS)).astype(np.float32)
    scores = -np.sum(max_lp, axis=1, keepdims=True, dtype=np.float32)

    prev = np.concatenate(
        [np.full((B, 1), -1, dtype=best.dtype), best[:, :-1]], axis=1
    )
    keep = (best != blank) & (best != prev)
    pos = np.cumsum(keep.astype(np.int64), axis=1) - 1
    pos = np.where(keep, pos, T)
    out = np.full((B, T + 1), -1, dtype=np.int32)
    out[np.arange(B)[:, None], pos] = best.astype(np.int32)
    decoded = out[:, :T]
    return decoded, scores.astype(np.float32)


def kernel(inputs):
    x = np.ascontiguousarray(np.asarray(inputs, dtype=np.float32))
    assert x.shape == (B, T, C)
    best, maxp, _ = _run_device(x)
    return _decode(best, maxp)


# revision 4
# speedup vs baseline: 1.3346x; 1.3346x over previous
"""CTC best-path decoder (beam_width=1) for Trainium2, 8 NeuronCores.

Heavy part (argmax + max over the 2500-class axis of [64,256,2500] softmax
probs) runs on device, data-parallel over the batch: each core gets 8
sequences = 2048 (b,t) rows. Per 128-row tile:
  1. stream [128,2500] f32 HBM->SBUF
  2. DVE reduce_max over 125 subchunks of 20 -> fm[128,125]  (the only
     full-rate scan the Vector engine does)
  3. DVE max (top-8) on fm -> row max value; max_index on fm -> winning
     subchunk cf
  4. GPSIMD ap_gather pulls, per 16-partition group, the 16 winning
     subchunks -> g[128,16,20]
  5. DVE max_index on g (320 elems) -> q; host maps q -> within-subchunk
     offset (q % 20), final argmax = cf*20 + q%20
Host verifies x[row, argmax] == rowmax (numpy re-argmax fallback on any
mismatch), then computes scores = -sum(log(maxp+eps)) and the cheap
[64,256] CTC collapse in numpy. log is monotone so argmax(p) ==
argmax(log(p+eps)) and max(log(p+eps)) == log(max(p)+eps) exactly.
"""

import numpy as np

import concourse.bacc as bacc
import concourse.mybir as mybir
import concourse.tile as tile
from concourse.bass_utils import run_bass_kernel_spmd

EPS = 1e-7

B, T, C = 64, 256, 2500
NCORES = 8
BLOC = B // NCORES          # sequences per core
ROWS = BLOC * T             # 2048 (b,t) rows per core
P = 128
NTILES = ROWS // P          # 16
D = 20                      # subchunk width
NSUB = C // D               # 125 subchunks per row


def build():
    nc = bacc.Bacc("TRN2", target_bir_lowering=False, debug=False)

    x = nc.declare_dram_parameter("x", [ROWS, C], mybir.dt.float32, isOutput=False)
    out_val = nc.declare_dram_parameter(
        "out_val", [P, NTILES * 8], mybir.dt.float32, isOutput=True
    )
    out_cf = nc.declare_dram_parameter(
        "out_cf", [P, NTILES * 8], mybir.dt.uint16, isOutput=True
    )
    out_q = nc.declare_dram_parameter(
        "out_q", [P, NTILES * 8], mybir.dt.uint16, isOutput=True
    )

    x_tiles = x[:].rearrange("(n p) c -> n p c", p=P)        # [16,128,2500]

    with tile.TileContext(nc) as tc:
        with (
            tc.tile_pool(name="xp", bufs=4) as xp,
            tc.tile_pool(name="sp", bufs=4) as sp,
            tc.tile_pool(name="acc", bufs=1) as accp,
        ):
            gm_all = accp.tile([P, NTILES * 8], mybir.dt.float32)
            cf_all = accp.tile([P, NTILES * 8], mybir.dt.uint16)
            q_all = accp.tile([P, NTILES * 8], mybir.dt.uint16)

            for t in range(NTILES):
                xt = xp.tile([P, C], mybir.dt.float32, tag="xt")
                nc.sync.dma_start(out=xt[:], in_=x_tiles[t])

                fm = sp.tile([P, NSUB], mybir.dt.float32, tag="fm")
                nc.vector.reduce_max(
                    out=fm[:],
                    in_=xt[:].rearrange("p (k g) -> p k g", g=D),
                    axis=mybir.AxisListType.X,
                )

                gm = gm_all[:, t * 8 : (t + 1) * 8]
                nc.vector.max(out=gm, in_=fm[:])
                cf = cf_all[:, t * 8 : (t + 1) * 8]
                nc.vector.max_index(out=cf, in_max=gm, in_values=fm[:])

                g = sp.tile([P, 16 * D], mybir.dt.float32, tag="g")
                nc.gpsimd.ap_gather(
                    out_ap=g[:],
                    in_ap=xt[:],
                    idxs_ap=cf_all[:, t * 8 : t * 8 + 1].bitcast(mybir.dt.int16),
                    channels=P,
                    num_elems=NSUB,
                    d=D,
                    num_idxs=16,
                )
                nc.vector.max_index(
                    out=q_all[:, t * 8 : (t + 1) * 8], in_max=gm, in_values=g[:]
                )

            nc.sync.dma_start(out=out_val[:], in_=gm_all[:])
            nc.sync.dma_start(out=out_cf[:], in_=cf_all[:])
            nc.sync.dma_start(out=out_q[:], in_=q_all[:])

    nc.finalize()
    return nc


def _run_device(x, trace=False):
    """x: [64,256,2500] f32 -> (best [64,256] int64, maxp [64,256] f32, results)"""
    shards = np.ascontiguousarray(x.reshape(NCORES, ROWS, C))
    nc = build()
    in_maps = [{"x": shards[i]} for i in range(NCORES)]
    res = run_bass_kernel_spmd(
        nc, in_maps, core_ids=list(range(NCORES)), trace=trace
    )
    best = np.empty((NCORES, ROWS), np.int64)
    maxp = np.empty((NCORES, ROWS), np.float32)
    for i in range(NCORES):
        gm = res.results[i]["out_val"][:, ::8]                  # [128,16]
        cf = res.results[i]["out_cf"][:, ::8].astype(np.int64)
        q = res.results[i]["out_q"][:, ::8].astype(np.int64)
        fi = cf * D + (q % D)
        maxp[i] = gm.T.reshape(ROWS)
        best[i] = fi.T.reshape(ROWS)
    best = best.reshape(B * T)
    maxp = maxp.reshape(B * T)

    # Safety: the device argmax must reproduce the device max value; any row
    # where it doesn't (ties across subchunks, unexpected layout issue) is
    # recomputed exactly on host.
    x2d = x.reshape(B * T, C)
    bad = x2d[np.arange(B * T), best] != maxp
    if bad.any():
        best[bad] = np.argmax(x2d[bad], axis=1)
    return best.reshape(B, T), maxp.reshape(B, T), res


def _decode(best, maxp):
    """CTC collapse + scores, numpy, matching K.ctc_decode semantics."""
    blank = C - 1
    max_lp = np.log(maxp + np.float32(EPS)).astype(np.float32)
    scores = -np.sum(max_lp, axis=1, keepdims=True, dtype=np.float32)

    prev = np.concatenate(
        [np.full((B, 1), -1, dtype=best.dtype), best[:, :-1]], axis=1
    )
    keep = (best != blank) & (best != prev)
    pos = np.cumsum(keep.astype(np.int64), axis=1) - 1
    pos = np.where(keep, pos, T)
    out = np.full((B, T + 1), -1, dtype=np.int32)
    out[np.arange(B)[:, None], pos] = best.astype(np.int32)
    decoded = out[:, :T]
    return decoded, scores.astype(np.float32)


def kernel(inputs):
    x = np.ascontiguousarray(np.asarray(inputs, dtype=np.float32))
    assert x.shape == (B, T, C)
    best, maxp, _ = _run_device(x)
    return _decode(best, maxp)
